# revision 45
# baseline (speedup 1.0000x reference)
"""Cached self-attention Trainium2 kernel (v4).

Sharding: 8 cores = 2 batches x 4 head-groups. Core c: batch b=c//4, group
g=c%4 owns heads 4g..4g+3 (columns 512g:512g+512 of the q/k/v projections).
Each core projects q/k/v for its heads over the full sequence, runs attention
for its 4 heads, the 4 cores of a batch AllGather the normalized per-head
attention outputs per 512-column q-block, and each core computes the output
projection onto its 512-column slice of wo, so outputs tile the model dim.

v4 changes vs v3 (baseline 738us):
- input DMAs reordered: xT + first weight chunk first, caches during phase 1
  (kills a 60us PE stall at kernel start)
- output projection (phase 3) for q-block sb is interleaved into the
  attention of sb+1, one 128-row chunk after each head, so the AllGather
  latency and the whole phase-3 tail are hidden (kills ~40us tail + HAM
  re-throttle)
- softmax z-chain restructured: fp16 broadcast matmul (was fp32, 704ns),
  finalize of head j emitted after head j+1's matmuls so the PE never waits
  on the DVE reciprocal (kills 0.9us stall + 4.1us HAM cold restart per
  head-block)
- all PSUM evacuations moved off the Scalar engine (to DVE) so ACT does only
  exp; q scaling folded into wq/bq host-side
- PSUM: pss [128,4,512] rotated in halves + PA double-buffered + shared
  psz/psb/psO tag = exactly 8 banks
"""
import numpy as np
from contextlib import ExitStack

import concourse.bass as bass
import concourse.tile as tile
from concourse import bacc, bass_isa, mybir
from concourse.bass_utils import run_bass_kernel_spmd
from concourse.tile_rust import add_dep_helper

B, S, PC, D, H = 2, 2048, 2048, 2048, 16
HD = D // H            # 128 head dim
GH = H // 4            # 4 heads per core
DG = GH * HD           # 512 head-dims per core
NB = 512               # q-block size
NKC = (PC + S) // HD   # 32 key chunks of 128
NDC = D // HD          # 16 contraction chunks
F16 = mybir.dt.float16
F32 = mybir.dt.float32
AF = mybir.ActivationFunctionType
ALU = mybir.AluOpType
INV_SQRT_HD = float(1.0 / np.sqrt(HD))

GROUPS = [[0, 1, 2, 3], [4, 5, 6, 7]]


def build():
    nc = bacc.Bacc("TRN2", target_bir_lowering=False, debug=False, num_devices=8)

    def inp(name, shape):
        return nc.dram_tensor(name, shape, F16, kind="ExternalInput").ap()

    xT = inp("xT", [D, S])          # x[b].T
    wq = inp("wq", [D, DG])         # wq[:, 512g:512g+512] / sqrt(HD)
    bq = inp("bq", [DG])            # bq slice / sqrt(HD)
    wk = inp("wk", [D, DG])
    bk = inp("bk", [DG])
    wv = inp("wv", [D, DG])
    bv = inp("bv", [DG])
    ckT = inp("ckT", [DG, PC])      # cache_k[b,:,slice].T
    cv = inp("cv", [PC, DG])        # cache_v[b,:,slice]
    wo = inp("wo", [D, DG])         # wo[:, 512g:512g+512] (natural rows)
    bo = inp("bo", [DG])
    y = nc.dram_tensor("y", [S, DG], F32, kind="ExternalOutput").ap()

    with tile.TileContext(nc) as tc, ExitStack() as ctx:
        res = ctx.enter_context(tc.tile_pool(name="res", bufs=1))
        dram = ctx.enter_context(tc.tile_pool(name="dram", bufs=1, space="DRAM"))

        # tiny whole-kernel residents
        bq_t = res.tile([HD, GH], F16, tag="bq")
        bk_t = res.tile([HD, GH], F16, tag="bk")
        bv_t = res.tile([1, DG], F16, tag="bv")
        bo_t = res.tile([1, DG], F16, tag="bo")
        ones_k = res.tile([HD, 1], F16, tag="ones_k")      # [128,1] ones
        ones_r16 = res.tile([1, HD], F16, tag="ones_r16")  # [1,128] ones
        bv_b = res.tile([HD, DG], F32, tag="bv_b")         # bv broadcast
        bo_b = res.tile([HD, DG], F32, tag="bo_b")         # bo broadcast
        dum = res.tile([1, 16], F32, tag="dum")
        dum_o = res.tile([1, 16], F16, tag="dum_o")
        nc.sync.dma_start(bq_t[:], bq.rearrange("(m p) -> p m", p=HD))
        nc.sync.dma_start(bk_t[:], bk.rearrange("(m p) -> p m", p=HD))
        nc.sync.dma_start(bv_t[:], bv[None, :])
        nc.sync.dma_start(bo_t[:], bo[None, :])
        ones_w = res.tile([HD, NB], F16, tag="ones_w")
        nc.vector.memset(ones_k[:], 1.0)
        nc.vector.memset(ones_r16[:], 1.0)
        nc.vector.memset(ones_w[:], 0.0)
        nc.vector.memset(dum[:], 0.0)
        # preload the exp table set before the first Identity/Copy activation
        nc.scalar.activation(dum_o[:], dum[:], AF.Exp)

        # collective bounce buffers: one per q-block for sb 0-2; two
        # head-pair halves for sb 3 so the tail exposes only a small gather
        bounce_in = []
        bounce_out = []
        for sb in range(3):
            bounce_in.append(dram.tile([HD, GH, NB], F16, tag=f"bi{sb}",
                                       name=f"bi{sb}"))
            bounce_out.append(dram.tile([4, HD, GH, NB], F16, tag=f"bg{sb}",
                                        name=f"bg{sb}"))
        b3i = [dram.tile([HD, 2, NB], F16, tag=f"b3i{h}", name=f"b3i{h}")
               for h in range(2)]
        b3g = [dram.tile([4, HD, 2, NB], F16, tag=f"b3g{h}", name=f"b3g{h}")
               for h in range(2)]
        # rank-sync collective (CC-path barrier before the phase-2 gathers)
        sync_i = dram.tile([1, 16], F16, tag="sync_i", name="sync_i")
        sync_o = dram.tile([4, 1, 16], F16, tag="sync_o", name="sync_o")

        # phase 1+2 residents
        ph = ctx.enter_context(tc.tile_pool(name="ph", bufs=1))
        qT = ph.tile([HD, GH, 4, NB], F16, tag="qT")       # [128, 4, 4, 512]
        kTn = ph.tile([HD, GH, 4, NB], F16, tag="kTn")
        ckT_t = ph.tile([HD, GH, PC], F16, tag="ckT")
        cv_t = ph.tile([HD, PC // HD, DG], F16, tag="cv")  # [128, 16, 512]
        vn_t = ph.tile([HD, S // HD, DG], F16, tag="vn")

        # ---- phase 1: projections ----
        with tc.tile_pool(name="px", bufs=1) as px, \
             tc.tile_pool(name="pw", bufs=2) as pw, \
             tc.tile_pool(name="ps1", bufs=1, space="PSUM") as ps1:
            # x first: the very first matmuls need it
            wt0 = pw.tile([HD, NDC, HD], F16, tag="wqk", name="wt0")
            nc.sync.dma_start(wt0[:], wq[:, 0:HD].rearrange(
                "(kc p) n -> p kc n", p=HD))
            xres = px.tile([HD, NDC, S], F16, tag="xres")   # 8.4 MB
            xr = xT.rearrange("(kc p) s -> p kc s", p=HD)
            for kq in range(8):
                nc.sync.dma_start(xres[:, 2 * kq:2 * (kq + 1), :],
                                  xr[:, 2 * kq:2 * (kq + 1), :])

            # fill the initial input-DMA wait with throwaway matmuls so the
            # PE clock is warm when the first projection matmuls arrive
            warm0 = ps1.tile([HD, NB], F32, tag="psq0", name="warm0")
            for t in range(20):
                nc.tensor.matmul(warm0[:], ones_w[:, 0:HD], ones_w[:],
                                 start=True, stop=True, skip_group_check=True)

            # bias broadcast rows via tiny rank-1 matmuls (PE idle anyway)
            psb_v = ps1.tile([HD, NB], F32, tag="psq0", name="psb_v")
            nc.tensor.matmul(psb_v[:], ones_r16[:], bv_t[:],
                             start=True, stop=True)
            nc.vector.tensor_copy(bv_b[:], psb_v[:])
            psb_o = ps1.tile([HD, NB], F32, tag="psq1", name="psb_o")
            nc.tensor.matmul(psb_o[:], ones_r16[:], bo_t[:],
                             start=True, stop=True)
            nc.vector.tensor_copy(bo_b[:], psb_o[:])

            # q pass then k pass; weights stay loaded across the 4 s-blocks
            for wi, (wsrc, dst, bias_t) in enumerate(
                    ((wq, qT, bq_t), (wk, kTn, bk_t))):
                for m in range(GH):
                    if wi == 0 and m == 0:
                        wt = wt0
                    else:
                        wt = pw.tile([HD, NDC, HD], F16, tag="wqk", name="wt")
                        nc.sync.dma_start(
                            wt[:], wsrc[:, HD * m:HD * (m + 1)].rearrange(
                                "(kc p) n -> p kc n", p=HD))
                    psq = ps1.tile([HD, 4, NB], F32,
                                   tag=f"psq{m % 2}", name="psq")
                    for kc in range(NDC):
                        for sb in range(4):
                            nc.tensor.matmul(
                                psq[:, sb, :], wt[:, kc, :],
                                xres[:, kc, NB * sb:NB * (sb + 1)],
                                start=(kc == 0), stop=(kc == NDC - 1))
                    nc.scalar.activation(dst[:, m], psq[:], AF.Identity,
                                         bias=bias_t[:, m:m + 1])
                if wi == 0:
                    # loads for later phases, behind the q-pass traffic
                    wvt = px.tile([HD, NDC, DG], F16, tag="wvt")
                    nc.sync.dma_start(
                        wvt[:], wv.rearrange("(kc p) n -> p kc n", p=HD))
                    nc.sync.dma_start(
                        ckT_t[:], ckT.rearrange("(m p) s -> p m s", p=HD))
                else:
                    nc.sync.dma_start(
                        cv_t[:], cv.rearrange("(ss p) d -> p ss d", p=HD))

            # v pass (natural layout)
            for ssg in range(4):
                psv = ps1.tile([HD, 4, NB], F32, tag=f"psq{ssg % 2}",
                               name="psv")
                for s4 in range(4):
                    ss = 4 * ssg + s4
                    for kc in range(NDC):
                        nc.tensor.matmul(psv[:, s4, :],
                                         xres[:, kc, HD * ss:HD * (ss + 1)],
                                         wvt[:, kc, :],
                                         start=(kc == 0), stop=(kc == NDC - 1))
                for s4 in range(4):
                    nc.vector.tensor_tensor(vn_t[:, 4 * ssg + s4, :],
                                            psv[:, s4, :], bv_b[:], ALU.add)

            # fill the phase-1 -> attention handoff (first exps wind up the
            # ACT/DVE pipeline) with throwaway matmuls gated on the v pass
            warm1 = ps1.tile([HD, NB], F32, tag="psq1", name="warm1")
            for t in range(16):
                nc.tensor.matmul(warm1[:], vn_t[:, 15, 0:HD],
                                 vn_t[:, 15, 0:NB],
                                 start=True, stop=True, skip_group_check=True)

        # CC-path rank synchronizer: a tiny AllGather queued before the real
        # gathers absorbs the cross-core skew accumulated during phase 1, so
        # the first real gather doesn't pay it
        nc.gpsimd.collective_compute(
            "AllGather", ALU.bypass, replica_groups=GROUPS,
            ins=[sync_i.opt()], outs=[sync_o.opt()])

        # ---- phase 2+3: attention + AllGather + interleaved out-proj ----
        with tc.tile_pool(name="wo3", bufs=1) as wop, \
             tc.tile_pool(name="p2", bufs=4) as p2, \
             tc.tile_pool(name="zp", bufs=2) as zp, \
             tc.tile_pool(name="ap", bufs=2) as apool, \
             tc.tile_pool(name="lt3", bufs=2) as ltp, \
             tc.tile_pool(name="p3", bufs=3) as p3p, \
             tc.tile_pool(name="pss", bufs=2, space="PSUM") as pssP, \
             tc.tile_pool(name="pa", bufs=2, space="PSUM") as paP, \
             tc.tile_pool(name="po", bufs=2, space="PSUM") as poP:
            wot = wop.tile([HD, 16, NB], F16, tag="wo")
            nc.sync.dma_start(wot[:], wo.rearrange("(c p) n -> p c n", p=HD))

            lts = [None] * 4

            def p3_mm(psO, src_sb, jj, t, anchor=None):
                g, j2 = divmod(t, 4)
                if src_sb == 3:
                    lt = lt3h[j2 // 2]
                    lhsT = lt[:, g, j2 % 2, HD * jj:HD * (jj + 1)]
                else:
                    lhsT = lts[src_sb][:, g, j2, HD * jj:HD * (jj + 1)]
                mm = nc.tensor.matmul(
                    psO[:], lhsT, wot[:, 4 * g + j2, :],
                    start=(t == 0), stop=(t == 15), skip_group_check=True)
                if anchor is not None:
                    # ordering-only dep: keep this chunk inside the head it
                    # was emitted for, so the scheduler cannot hoist it into
                    # an earlier PE hole where its lt load hasn't landed
                    add_dep_helper(mm.ins, anchor.ins, sync=False,
                                   reason="pin p3 chunk after its head start")

            def p3_evac(psO, src_sb, jj):
                m = 4 * src_sb + jj
                ot = p3p.tile([HD, NB], F32, tag="ot")
                nc.vector.tensor_tensor(ot[:], psO[:], bo_b[:], ALU.add)
                nc.sync.dma_start(y[HD * m:HD * (m + 1), :], ot[:])

            def p3_chunk(src_sb, jj, anchor=None):
                psO = poP.tile([HD, NB], F32, tag="po", name="psO")
                for t in range(16):
                    p3_mm(psO, src_sb, jj, t, anchor=anchor if t == 0 else None)
                p3_evac(psO, src_sb, jj)

            pending_fin = None   # (sb, j, PA, zbinv, ahead)
            lt3h = [None, None]  # sb-3 half-gather tiles (heads 01 / 23)

            def finalize(fin):
                sb_, j, PA, zbinv, ahead = fin
                nc.vector.tensor_tensor(ahead[:, j, :], PA[:], zbinv[:],
                                        ALU.mult)
                if sb_ == 3 and j in (1, 3):
                    # half-gather of the last q-block (heads 01 / heads 23):
                    # the first half also re-syncs the replica group so the
                    # final exposed gather is small and skew-free
                    h = j // 2
                    nc.sync.dma_start(b3i[h][:], ahead[:, 2 * h:2 * h + 2, :])
                    nc.gpsimd.collective_compute(
                        "AllGather", ALU.bypass, replica_groups=GROUPS,
                        ins=[b3i[h].opt()], outs=[b3g[h].opt()])
                    lt = ltp.tile([HD, 4, 2, NB], F16, tag=f"lth{h}",
                                  name=f"lth{h}")
                    for r in range(4):
                        nc.sync.dma_start(lt[:, r, :, :], b3g[h][r])
                    lt3h[h] = lt

            for sb in range(4):
                ahead = apool.tile([HD, GH, NB], F16, tag="ah")
                for j in range(GH):
                    scope = nc.named_scope(f"s{sb}h{j}")
                    scope.__enter__()
                    qTs = qT[:, j, sb, :]
                    PA = paP.tile([HD, NB], F32, tag="PA", name="PA")
                    zacc2 = zp.tile([HD, 2, NB], F16, tag="z")
                    head_anchor = None
                    for g in range(16):
                        pss = pssP.tile([HD, 2, NB], F32, tag="pss",
                                        name="pss")
                        e2 = p2.tile([HD, 2, NB], F16, tag="e")
                        for i in range(2):
                            c = 2 * g + i
                            if c < PC // HD:
                                kt = ckT_t[:, j, HD * c:HD * (c + 1)]
                            else:
                                cc = c - PC // HD
                                kt = kTn[:, j, cc // 4,
                                         HD * (cc % 4):HD * (cc % 4 + 1)]
                            mm = nc.tensor.matmul(pss[:, i, :], kt, qTs,
                                                  start=True, stop=True)
                            if head_anchor is None:
                                head_anchor = mm
                        nc.scalar.activation(e2[:], pss[:], AF.Exp)
                        for i in range(2):
                            c = 2 * g + i
                            if c < PC // HD:
                                vt = cv_t[:, c, HD * j:HD * (j + 1)]
                            else:
                                vt = vn_t[:, c - PC // HD,
                                          HD * j:HD * (j + 1)]
                            nc.tensor.matmul(PA[:], vt, e2[:, i, :],
                                             start=(c == 0),
                                             stop=(c == NKC - 1),
                                             skip_group_check=True)
                        if g == 0:
                            nc.vector.tensor_copy(zacc2[:], e2[:])
                        else:
                            nc.vector.tensor_tensor(zacc2[:], zacc2[:],
                                                    e2[:], ALU.add)
                    # head tail: cross-partition Z on GpSimd, 1/Z on DVE
                    zs = zp.tile([HD, NB], F32, tag="zs")
                    nc.vector.tensor_tensor(zs[:], zacc2[:, 0, :],
                                            zacc2[:, 1, :], ALU.add)
                    zred = zp.tile([HD, NB], F32, tag="zr")
                    nc.gpsimd.partition_all_reduce(zred[:], zs[:], HD,
                                                   bass_isa.ReduceOp.add)
                    zbinv = zp.tile([HD, NB], F32, tag="zbi")
                    nc.vector.reciprocal_approx_fast(zbinv[:], zred[:])
                    scope.__exit__(None, None, None)

                    # out-projection chunks, scheduled 6 heads behind their
                    # gather (~44us+ cushion) so collective skew/contention
                    # can never stall the in-order PE queue
                    n = 4 * sb + j
                    if n >= 6 and n - 6 < 10:
                        c = n - 6
                        p3_chunk(c // 4, c % 4, anchor=head_anchor)

                    # deferred finalize of the previous head; by now its
                    # 1/Z has long completed, so the PA multiply is off the
                    # critical path
                    if pending_fin is not None:
                        finalize(pending_fin)
                    pending_fin = (sb, j, PA, zbinv, ahead)

                # end of q-block: finalize last head, gather (sb3 gathers in
                # halves inside finalize instead)
                finalize(pending_fin)
                pending_fin = None
                if sb < 3:
                    nc.sync.dma_start(bounce_in[sb][:], ahead[:])
                    nc.gpsimd.collective_compute(
                        "AllGather", ALU.bypass, replica_groups=GROUPS,
                        ins=[bounce_in[sb].opt()], outs=[bounce_out[sb].opt()])
                    lt = ltp.tile([HD, 4, GH, NB], F16, tag="lt", name="lt")
                    for r in range(4):
                        nc.sync.dma_start(lt[:, r, :, :], bounce_out[sb][r])
                    lts[sb] = lt

            # tail: remaining sb2 chunks (ready — they cover the final
            # half-gather wait), warm-up matmuls to hold the PE clock, then
            # the out-projection of the last q-block
            scope = nc.named_scope("tail")
            scope.__enter__()
            p3_chunk(2, 2)
            p3_chunk(2, 3)
            warm = paP.tile([HD, NB], F32, tag="PA", name="warm")
            for t in range(30):
                nc.tensor.matmul(warm[:], ahead[:, 3, 0:HD], ahead[:, 3, :],
                                 start=True, stop=True, skip_group_check=True)
            for jj in range(4):
                p3_chunk(3, jj)
            scope.__exit__(None, None, None)

    nc.compile()
    return nc


_BUILT = None


def get_built():
    global _BUILT
    if _BUILT is None:
        _BUILT = build()
    return _BUILT


def make_in_maps(x, cache_k, cache_v, wq, bq, wk, bk, wv, bv, wo, bo):
    x = np.asarray(x)
    cache_k = np.asarray(cache_k)
    cache_v = np.asarray(cache_v)
    wq, bq = np.asarray(wq), np.asarray(bq)
    wk, bk = np.asarray(wk), np.asarray(bk)
    wv, bv = np.asarray(wv), np.asarray(bv)
    wo, bo = np.asarray(wo), np.asarray(bo)

    in_maps = []
    for c in range(8):
        b, g = divmod(c, 4)
        sl = slice(DG * g, DG * (g + 1))
        in_maps.append({
            "xT": np.ascontiguousarray(x[b].T).astype(np.float16),
            "wq": (wq[:, sl] * INV_SQRT_HD).astype(np.float16),
            "bq": (bq[sl] * INV_SQRT_HD).astype(np.float16),
            "wk": wk[:, sl].astype(np.float16),
            "bk": bk[sl].astype(np.float16),
            "wv": wv[:, sl].astype(np.float16),
            "bv": bv[sl].astype(np.float16),
            "ckT": np.ascontiguousarray(cache_k[b][:, sl].T).astype(np.float16),
            "cv": cache_v[b][:, sl].astype(np.float16),
            "wo": wo[:, sl].astype(np.float16),
            "bo": bo[sl].astype(np.float16),
        })
    return in_maps


def assemble(results):
    out = np.empty((B, S, D), np.float32)
    for c in range(8):
        b, g = divmod(c, 4)
        out[b, :, DG * g:DG * (g + 1)] = results[c]["y"]
    return out


def kernel(**inputs):
    nc = get_built()
    in_maps = make_in_maps(**inputs)
    res = run_bass_kernel_spmd(nc, in_maps, core_ids=list(range(8)))
    return assemble(res.results)


# revision 46
# speedup vs baseline: 1.0074x; 1.0074x over previous
"""Cached self-attention Trainium2 kernel (v4).

Sharding: 8 cores = 2 batches x 4 head-groups. Core c: batch b=c//4, group
g=c%4 owns heads 4g..4g+3 (columns 512g:512g+512 of the q/k/v projections).
Each core projects q/k/v for its heads over the full sequence, runs attention
for its 4 heads, the 4 cores of a batch AllGather the normalized per-head
attention outputs per 512-column q-block, and each core computes the output
projection onto its 512-column slice of wo, so outputs tile the model dim.

v4 changes vs v3 (baseline 738us):
- input DMAs reordered: xT + first weight chunk first, caches during phase 1
  (kills a 60us PE stall at kernel start)
- output projection (phase 3) for q-block sb is interleaved into the
  attention of sb+1, one 128-row chunk after each head, so the AllGather
  latency and the whole phase-3 tail are hidden (kills ~40us tail + HAM
  re-throttle)
- softmax z-chain restructured: fp16 broadcast matmul (was fp32, 704ns),
  finalize of head j emitted after head j+1's matmuls so the PE never waits
  on the DVE reciprocal (kills 0.9us stall + 4.1us HAM cold restart per
  head-block)
- all PSUM evacuations moved off the Scalar engine (to DVE) so ACT does only
  exp; q scaling folded into wq/bq host-side
- PSUM: pss [128,4,512] rotated in halves + PA double-buffered + shared
  psz/psb/psO tag = exactly 8 banks
"""
import numpy as np
from contextlib import ExitStack

import concourse.bass as bass
import concourse.tile as tile
from concourse import bacc, bass_isa, mybir
from concourse.bass_utils import run_bass_kernel_spmd
from concourse.tile_rust import add_dep_helper

B, S, PC, D, H = 2, 2048, 2048, 2048, 16
HD = D // H            # 128 head dim
GH = H // 4            # 4 heads per core
DG = GH * HD           # 512 head-dims per core
NB = 512               # q-block size
NKC = (PC + S) // HD   # 32 key chunks of 128
NDC = D // HD          # 16 contraction chunks
F16 = mybir.dt.float16
F32 = mybir.dt.float32
AF = mybir.ActivationFunctionType
ALU = mybir.AluOpType
INV_SQRT_HD = float(1.0 / np.sqrt(HD))

GROUPS = [[0, 1, 2, 3], [4, 5, 6, 7]]


def build():
    nc = bacc.Bacc("TRN2", target_bir_lowering=False, debug=False, num_devices=8)

    def inp(name, shape):
        return nc.dram_tensor(name, shape, F16, kind="ExternalInput").ap()

    xT = inp("xT", [D, S])          # x[b].T
    wq = inp("wq", [D, DG])         # wq[:, 512g:512g+512] / sqrt(HD)
    bq = inp("bq", [DG])            # bq slice / sqrt(HD)
    wk = inp("wk", [D, DG])
    bk = inp("bk", [DG])
    wv = inp("wv", [D, DG])
    bv = inp("bv", [DG])
    ckT = inp("ckT", [DG, PC])      # cache_k[b,:,slice].T
    cv = inp("cv", [PC, DG])        # cache_v[b,:,slice]
    wo = inp("wo", [D, DG])         # wo[:, 512g:512g+512] (natural rows)
    bo = inp("bo", [DG])
    y = nc.dram_tensor("y", [S, DG], F32, kind="ExternalOutput").ap()

    with tile.TileContext(nc) as tc, ExitStack() as ctx:
        res = ctx.enter_context(tc.tile_pool(name="res", bufs=1))
        dram = ctx.enter_context(tc.tile_pool(name="dram", bufs=1, space="DRAM"))

        # tiny whole-kernel residents
        bq_t = res.tile([HD, GH], F16, tag="bq")
        bk_t = res.tile([HD, GH], F16, tag="bk")
        bv_t = res.tile([1, DG], F16, tag="bv")
        bo_t = res.tile([1, DG], F16, tag="bo")
        ones_k = res.tile([HD, 1], F16, tag="ones_k")      # [128,1] ones
        ones_r16 = res.tile([1, HD], F16, tag="ones_r16")  # [1,128] ones
        bv_b = res.tile([HD, DG], F32, tag="bv_b")         # bv broadcast
        bo_b = res.tile([HD, DG], F32, tag="bo_b")         # bo broadcast
        dum = res.tile([1, 16], F32, tag="dum")
        dum_o = res.tile([1, 16], F16, tag="dum_o")
        nc.sync.dma_start(bq_t[:], bq.rearrange("(m p) -> p m", p=HD))
        nc.sync.dma_start(bk_t[:], bk.rearrange("(m p) -> p m", p=HD))
        nc.sync.dma_start(bv_t[:], bv[None, :])
        nc.sync.dma_start(bo_t[:], bo[None, :])
        nc.vector.memset(ones_k[:], 1.0)
        nc.vector.memset(ones_r16[:], 1.0)
        nc.vector.memset(dum[:], 0.0)
        # preload the exp table set before the first Identity/Copy activation
        nc.scalar.activation(dum_o[:], dum[:], AF.Exp)

        # collective bounce buffers: one per q-block for sb 0-2; two
        # head-pair halves for sb 3 so the tail exposes only a small gather
        bounce_in = []
        bounce_out = []
        for sb in range(3):
            bounce_in.append(dram.tile([HD, GH, NB], F16, tag=f"bi{sb}",
                                       name=f"bi{sb}"))
            bounce_out.append(dram.tile([4, HD, GH, NB], F16, tag=f"bg{sb}",
                                        name=f"bg{sb}"))
        b3i = [dram.tile([HD, 2, NB], F16, tag=f"b3i{h}", name=f"b3i{h}")
               for h in range(2)]
        b3g = [dram.tile([4, HD, 2, NB], F16, tag=f"b3g{h}", name=f"b3g{h}")
               for h in range(2)]
        # rank-sync collective (CC-path barrier before the phase-2 gathers)
        sync_i = dram.tile([1, 16], F16, tag="sync_i", name="sync_i")
        sync_o = dram.tile([4, 1, 16], F16, tag="sync_o", name="sync_o")

        # phase 1+2 residents
        ph = ctx.enter_context(tc.tile_pool(name="ph", bufs=1))
        qT = ph.tile([HD, GH, 4, NB], F16, tag="qT")       # [128, 4, 4, 512]
        kTn = ph.tile([HD, GH, 4, NB], F16, tag="kTn")
        ckT_t = ph.tile([HD, GH, PC], F16, tag="ckT")
        cv_t = ph.tile([HD, PC // HD, DG], F16, tag="cv")  # [128, 16, 512]
        vn_t = ph.tile([HD, S // HD, DG], F16, tag="vn")

        # ---- phase 1: projections ----
        with tc.tile_pool(name="px", bufs=1) as px, \
             tc.tile_pool(name="pw", bufs=2) as pw, \
             tc.tile_pool(name="ps1", bufs=1, space="PSUM") as ps1:
            # x first: the very first matmuls need it
            wt0 = pw.tile([HD, NDC, HD], F16, tag="wqk", name="wt0")
            nc.sync.dma_start(wt0[:], wq[:, 0:HD].rearrange(
                "(kc p) n -> p kc n", p=HD))
            xres = px.tile([HD, NDC, S], F16, tag="xres")   # 8.4 MB
            xr = xT.rearrange("(kc p) s -> p kc s", p=HD)
            for kq in range(8):
                nc.sync.dma_start(xres[:, 2 * kq:2 * (kq + 1), :],
                                  xr[:, 2 * kq:2 * (kq + 1), :])

            # bias broadcast rows via tiny rank-1 matmuls (PE idle anyway)
            psb_v = ps1.tile([HD, NB], F32, tag="psq0", name="psb_v")
            nc.tensor.matmul(psb_v[:], ones_r16[:], bv_t[:],
                             start=True, stop=True)
            nc.vector.tensor_copy(bv_b[:], psb_v[:])
            psb_o = ps1.tile([HD, NB], F32, tag="psq1", name="psb_o")
            nc.tensor.matmul(psb_o[:], ones_r16[:], bo_t[:],
                             start=True, stop=True)
            nc.vector.tensor_copy(bo_b[:], psb_o[:])

            # q pass then k pass; weights stay loaded across the 4 s-blocks
            for wi, (wsrc, dst, bias_t) in enumerate(
                    ((wq, qT, bq_t), (wk, kTn, bk_t))):
                for m in range(GH):
                    if wi == 0 and m == 0:
                        wt = wt0
                    else:
                        wt = pw.tile([HD, NDC, HD], F16, tag="wqk", name="wt")
                        nc.sync.dma_start(
                            wt[:], wsrc[:, HD * m:HD * (m + 1)].rearrange(
                                "(kc p) n -> p kc n", p=HD))
                    psq = ps1.tile([HD, 4, NB], F32,
                                   tag=f"psq{m % 2}", name="psq")
                    for kc in range(NDC):
                        for sb in range(4):
                            nc.tensor.matmul(
                                psq[:, sb, :], wt[:, kc, :],
                                xres[:, kc, NB * sb:NB * (sb + 1)],
                                start=(kc == 0), stop=(kc == NDC - 1))
                    nc.scalar.activation(dst[:, m], psq[:], AF.Identity,
                                         bias=bias_t[:, m:m + 1])
                if wi == 0:
                    # loads for later phases, behind the q-pass traffic
                    wvt = px.tile([HD, NDC, DG], F16, tag="wvt")
                    nc.sync.dma_start(
                        wvt[:], wv.rearrange("(kc p) n -> p kc n", p=HD))
                    nc.sync.dma_start(
                        ckT_t[:], ckT.rearrange("(m p) s -> p m s", p=HD))
                else:
                    nc.sync.dma_start(
                        cv_t[:], cv.rearrange("(ss p) d -> p ss d", p=HD))

            # v pass (natural layout)
            for ssg in range(4):
                psv = ps1.tile([HD, 4, NB], F32, tag=f"psq{ssg % 2}",
                               name="psv")
                for s4 in range(4):
                    ss = 4 * ssg + s4
                    for kc in range(NDC):
                        nc.tensor.matmul(psv[:, s4, :],
                                         xres[:, kc, HD * ss:HD * (ss + 1)],
                                         wvt[:, kc, :],
                                         start=(kc == 0), stop=(kc == NDC - 1))
                for s4 in range(4):
                    nc.vector.tensor_tensor(vn_t[:, 4 * ssg + s4, :],
                                            psv[:, s4, :], bv_b[:], ALU.add)

        # CC-path rank synchronizer: a tiny AllGather queued before the real
        # gathers absorbs the cross-core skew accumulated during phase 1, so
        # the first real gather doesn't pay it
        nc.gpsimd.collective_compute(
            "AllGather", ALU.bypass, replica_groups=GROUPS,
            ins=[sync_i.opt()], outs=[sync_o.opt()])

        # ---- phase 2+3: attention + AllGather + interleaved out-proj ----
        with tc.tile_pool(name="wo3", bufs=1) as wop, \
             tc.tile_pool(name="p2", bufs=4) as p2, \
             tc.tile_pool(name="zp", bufs=2) as zp, \
             tc.tile_pool(name="ap", bufs=2) as apool, \
             tc.tile_pool(name="lt3", bufs=2) as ltp, \
             tc.tile_pool(name="p3", bufs=3) as p3p, \
             tc.tile_pool(name="pss", bufs=2, space="PSUM") as pssP, \
             tc.tile_pool(name="pa", bufs=2, space="PSUM") as paP, \
             tc.tile_pool(name="po", bufs=2, space="PSUM") as poP:
            wot = wop.tile([HD, 16, NB], F16, tag="wo")
            nc.sync.dma_start(wot[:], wo.rearrange("(c p) n -> p c n", p=HD))

            lts = [None] * 4

            def p3_mm(psO, src_sb, jj, t, anchor=None):
                g, j2 = divmod(t, 4)
                if src_sb == 3:
                    lt = lt3h[j2 // 2]
                    lhsT = lt[:, g, j2 % 2, HD * jj:HD * (jj + 1)]
                else:
                    lhsT = lts[src_sb][:, g, j2, HD * jj:HD * (jj + 1)]
                mm = nc.tensor.matmul(
                    psO[:], lhsT, wot[:, 4 * g + j2, :],
                    start=(t == 0), stop=(t == 15), skip_group_check=True)
                if anchor is not None:
                    # ordering-only dep: keep this chunk inside the head it
                    # was emitted for, so the scheduler cannot hoist it into
                    # an earlier PE hole where its lt load hasn't landed
                    add_dep_helper(mm.ins, anchor.ins, sync=False,
                                   reason="pin p3 chunk after its head start")

            def p3_evac(psO, src_sb, jj):
                m = 4 * src_sb + jj
                ot = p3p.tile([HD, NB], F32, tag="ot")
                nc.vector.tensor_tensor(ot[:], psO[:], bo_b[:], ALU.add)
                nc.sync.dma_start(y[HD * m:HD * (m + 1), :], ot[:])

            def p3_chunk(src_sb, jj, anchor=None):
                psO = poP.tile([HD, NB], F32, tag="po", name="psO")
                for t in range(16):
                    p3_mm(psO, src_sb, jj, t, anchor=anchor if t == 0 else None)
                p3_evac(psO, src_sb, jj)

            pending_fin = None   # (sb, j, PA, zbinv, ahead)
            lt3h = [None, None]  # sb-3 half-gather tiles (heads 01 / 23)

            def finalize(fin):
                sb_, j, PA, zbinv, ahead = fin
                nc.vector.tensor_tensor(ahead[:, j, :], PA[:], zbinv[:],
                                        ALU.mult)
                if sb_ == 3 and j in (1, 3):
                    # half-gather of the last q-block (heads 01 / heads 23):
                    # the first half also re-syncs the replica group so the
                    # final exposed gather is small and skew-free
                    h = j // 2
                    nc.sync.dma_start(b3i[h][:], ahead[:, 2 * h:2 * h + 2, :])
                    nc.gpsimd.collective_compute(
                        "AllGather", ALU.bypass, replica_groups=GROUPS,
                        ins=[b3i[h].opt()], outs=[b3g[h].opt()])
                    lt = ltp.tile([HD, 4, 2, NB], F16, tag=f"lth{h}",
                                  name=f"lth{h}")
                    for r in range(4):
                        nc.sync.dma_start(lt[:, r, :, :], b3g[h][r])
                    lt3h[h] = lt

            for sb in range(4):
                ahead = apool.tile([HD, GH, NB], F16, tag="ah")
                for j in range(GH):
                    scope = nc.named_scope(f"s{sb}h{j}")
                    scope.__enter__()
                    qTs = qT[:, j, sb, :]
                    PA = paP.tile([HD, NB], F32, tag="PA", name="PA")
                    zacc2 = zp.tile([HD, 2, NB], F16, tag="z")
                    head_anchor = None
                    for g in range(16):
                        pss = pssP.tile([HD, 2, NB], F32, tag="pss",
                                        name="pss")
                        e2 = p2.tile([HD, 2, NB], F16, tag="e")
                        for i in range(2):
                            c = 2 * g + i
                            if c < PC // HD:
                                kt = ckT_t[:, j, HD * c:HD * (c + 1)]
                            else:
                                cc = c - PC // HD
                                kt = kTn[:, j, cc // 4,
                                         HD * (cc % 4):HD * (cc % 4 + 1)]
                            mm = nc.tensor.matmul(pss[:, i, :], kt, qTs,
                                                  start=True, stop=True)
                            if head_anchor is None:
                                head_anchor = mm
                        nc.scalar.activation(e2[:], pss[:], AF.Exp)
                        for i in range(2):
                            c = 2 * g + i
                            if c < PC // HD:
                                vt = cv_t[:, c, HD * j:HD * (j + 1)]
                            else:
                                vt = vn_t[:, c - PC // HD,
                                          HD * j:HD * (j + 1)]
                            nc.tensor.matmul(PA[:], vt, e2[:, i, :],
                                             start=(c == 0),
                                             stop=(c == NKC - 1),
                                             skip_group_check=True)
                        if g == 0:
                            nc.vector.tensor_copy(zacc2[:], e2[:])
                        else:
                            nc.vector.tensor_tensor(zacc2[:], zacc2[:],
                                                    e2[:], ALU.add)
                    # head tail: cross-partition Z on GpSimd, 1/Z on DVE
                    zs = zp.tile([HD, NB], F32, tag="zs")
                    nc.vector.tensor_tensor(zs[:], zacc2[:, 0, :],
                                            zacc2[:, 1, :], ALU.add)
                    zred = zp.tile([HD, NB], F32, tag="zr")
                    nc.gpsimd.partition_all_reduce(zred[:], zs[:], HD,
                                                   bass_isa.ReduceOp.add)
                    zbinv = zp.tile([HD, NB], F32, tag="zbi")
                    nc.vector.reciprocal_approx_fast(zbinv[:], zred[:])
                    scope.__exit__(None, None, None)

                    # out-projection chunks, scheduled 6 heads behind their
                    # gather (~44us+ cushion) so collective skew/contention
                    # can never stall the in-order PE queue
                    n = 4 * sb + j
                    if n >= 6 and n - 6 < 10:
                        c = n - 6
                        p3_chunk(c // 4, c % 4, anchor=head_anchor)

                    # deferred finalize of the previous head; by now its
                    # 1/Z has long completed, so the PA multiply is off the
                    # critical path
                    if pending_fin is not None:
                        finalize(pending_fin)
                    pending_fin = (sb, j, PA, zbinv, ahead)

                # end of q-block: finalize last head, gather (sb3 gathers in
                # halves inside finalize instead)
                finalize(pending_fin)
                pending_fin = None
                if sb < 3:
                    nc.sync.dma_start(bounce_in[sb][:], ahead[:])
                    nc.gpsimd.collective_compute(
                        "AllGather", ALU.bypass, replica_groups=GROUPS,
                        ins=[bounce_in[sb].opt()], outs=[bounce_out[sb].opt()])
                    lt = ltp.tile([HD, 4, GH, NB], F16, tag="lt", name="lt")
                    for r in range(4):
                        nc.sync.dma_start(lt[:, r, :, :], bounce_out[sb][r])
                    lts[sb] = lt

            # tail: remaining sb2 chunks (ready — they cover the final
            # half-gather wait), warm-up matmuls to hold the PE clock, then
            # the out-projection of the last q-block
            scope = nc.named_scope("tail")
            scope.__enter__()
            p3_chunk(2, 2)
            p3_chunk(2, 3)
            warm = paP.tile([HD, NB], F32, tag="PA", name="warm")
            for t in range(30):
                nc.tensor.matmul(warm[:], ahead[:, 3, 0:HD], ahead[:, 3, :],
                                 start=True, stop=True, skip_group_check=True)
            for jj in range(4):
                p3_chunk(3, jj)
            scope.__exit__(None, None, None)

    nc.compile()
    return nc


_BUILT = None


def get_built():
    global _BUILT
    if _BUILT is None:
        _BUILT = build()
    return _BUILT


def make_in_maps(x, cache_k, cache_v, wq, bq, wk, bk, wv, bv, wo, bo):
    x = np.asarray(x)
    cache_k = np.asarray(cache_k)
    cache_v = np.asarray(cache_v)
    wq, bq = np.asarray(wq), np.asarray(bq)
    wk, bk = np.asarray(wk), np.asarray(bk)
    wv, bv = np.asarray(wv), np.asarray(bv)
    wo, bo = np.asarray(wo), np.asarray(bo)

    in_maps = []
    for c in range(8):
        b, g = divmod(c, 4)
        sl = slice(DG * g, DG * (g + 1))
        in_maps.append({
            "xT": np.ascontiguousarray(x[b].T).astype(np.float16),
            "wq": (wq[:, sl] * INV_SQRT_HD).astype(np.float16),
            "bq": (bq[sl] * INV_SQRT_HD).astype(np.float16),
            "wk": wk[:, sl].astype(np.float16),
            "bk": bk[sl].astype(np.float16),
            "wv": wv[:, sl].astype(np.float16),
            "bv": bv[sl].astype(np.float16),
            "ckT": np.ascontiguousarray(cache_k[b][:, sl].T).astype(np.float16),
            "cv": cache_v[b][:, sl].astype(np.float16),
            "wo": wo[:, sl].astype(np.float16),
            "bo": bo[sl].astype(np.float16),
        })
    return in_maps


def assemble(results):
    out = np.empty((B, S, D), np.float32)
    for c in range(8):
        b, g = divmod(c, 4)
        out[b, :, DG * g:DG * (g + 1)] = results[c]["y"]
    return out


def kernel(**inputs):
    nc = get_built()
    in_maps = make_in_maps(**inputs)
    res = run_bass_kernel_spmd(nc, in_maps, core_ids=list(range(8)))
    return assemble(res.results)
